# revision 28
# baseline (speedup 1.0000x reference)
"""Trainium2 Bass kernel for MemoryEfficientAttnBlock3D.

Computes: y = x + conv1x1(attn(conv1x1_{q,k,v}(groupnorm(x))), wp, bp)
for x of shape (2, 64, 32, 32, 8)  (B=2, C=64, N=8192 tokens per batch).

Sharding: 8 cores = 2 batches x 4 query-chunks of 2048 tokens.  Each core
receives its batch's full token volume ROTATED so that its query chunk is
always tokens [0:2048] -- groupnorm statistics and softmax/AV reductions are
permutation-invariant over kv tokens, so all cores run an identical program.

Algebraic folds done on the host:
  - gamma folds into Wq/Wk/Wv columns; the attention scale 1/sqrt(C) into Wq.
  - k's additive constant (Wk@beta + bk) shifts every score of a softmax row
    equally -> dropped exactly (softmax shift invariance).
  - bq enters via a shift of the normalized activations: xh' = xn + c with
    Wq_eff@c = bq_eff.  q picks up bq exactly; k picks up a per-q-row score
    shift (softmax-invariant, cancels between numerator and denominator);
    v picks up Wpv@c which is subtracted from the output bias.  (For this
    problem bq_eff == 0 so c == 0.)
  - the output projection wp folds into the v weights (wp@Wv_eff), so the AV
    matmul accumulates wp@AV directly.
  - v's additive constant and bp are applied as a per-partition scalar in
    the final fused (t + bp) + x DVE op.

Tensor-engine packing (the attention matmuls only use half the 128x128 PE
array naively: K=64 contraction for scores, M=64 output for AV):
  - scores are ROW-TILED 2x: kv tiles are packed in pairs into k2
    [128, n/2] (tile 2p on partitions 0:64, tile 2p+1 on 64:128) and q is
    duplicated into both partition halves; two concurrent K=64 matmuls
    (tile_position (0,0) and (64,0)) produce two score tiles per 256-column
    stream.
  - AV keeps the proven 65-wide form (vt carries a ones column whose
    output row accumulates the softmax denominators); col-tiled partition-
    offset PSUM outputs are not lowered correctly by this stack (measured:
    device fault + CoreSim zero-region corruption), so M=65 stays.
  - projections stack the fp16 hi/lo weight split along the contraction
    (xh duplicated into both halves), halving matmul count vs two
    accumulating passes.

The PE runs at the cold 1.2 GHz p-state for the first ~117us of every
execution (power ramp; activity does NOT lift it early -- measured), so PE
work is minimized to keep the kernel ACT-bound: the ~16.8M softmax exps per
core on the Scalar engine (1 elem/lane/cycle @ 1.2 GHz, dtype-independent)
are the hard floor (~110us).
"""

import numpy as np

import concourse.bass as bass
import concourse.tile as tile
from concourse import bacc, bass_isa, mybir

F32 = mybir.dt.float32
F16 = mybir.dt.float16
AF = mybir.ActivationFunctionType
OP = mybir.AluOpType

C = 64
GROUPS = 32
EPS = 1e-6

B_FULL = 2
H_FULL, W_FULL, D_FULL = 32, 32, 8
N_FULL = H_FULL * W_FULL * D_FULL  # 8192 kv tokens per batch
N_CORES = 8
Q_CHUNKS = 4
M_FULL = N_FULL // Q_CHUNKS  # 2048 q tokens per core

MB = 256        # q-token block
NT = 128        # kv-token tile (matmul M / partition dim)
GSZ = 6         # kv tiles per exp group ([128, 1536] PSUM tile = 3 banks)
STAT_CHUNK = 512
PCH = 512       # projection chunk (tokens)


def emit(tc, nc, n_tok, m_tok, xb_d, wq4_d, wk4_d, wv2_d, bpc_d, c2_d,
         pair_d, expand2_d, out_d):
    ntiles = n_tok // NT          # 64
    npairs = ntiles // 2          # 32
    ngroups = (ntiles + GSZ - 1) // GSZ  # 11
    nch = max(1, n_tok // STAT_CHUNK)
    sch = n_tok // nch
    nkch = n_tok // PCH           # 16 k-projection chunks
    nvb = ntiles // 8             # 8 vt-projection batches
    xch = max(1, n_tok // 2048)   # 4 normalize macro-chunks
    xsz = n_tok // xch
    nblk = m_tok // MB            # 8

    def gsize(g):
        return min(GSZ, ntiles - g * GSZ)

    with (
        tc.tile_pool(name="persist", bufs=1) as persist,
        tc.tile_pool(name="expS", bufs=4) as epool,
        tc.tile_pool(name="mtail", bufs=3) as mpool,
        tc.tile_pool(name="spsum", bufs=2, space="PSUM") as spool,
        tc.tile_pool(name="avpsum", bufs=1, space="PSUM") as avpool,
        tc.tile_pool(name="prodp", bufs=1, space="PSUM") as prodpool,
        tc.tile_pool(name="dram", bufs=2, space="DRAM") as dpool,
    ):
        # ---- persistent SBUF tensors ----
        xb2_sb = persist.tile([2 * C, n_tok], F32)
        xh2_sb = persist.tile([2 * C, n_tok], F16)   # normalized, dup halves
        k2_sb = persist.tile([2 * C, n_tok // 2], F16)  # kv tile pairs
        q2_sb = persist.tile([2 * C, m_tok], F16)    # dup halves
        vt_sb = persist.tile([NT, ntiles * (C + 1)], F16)
        wq4_sb = persist.tile([2 * C, 2 * C], F16)
        wk4_sb = persist.tile([2 * C, 2 * C], F16)
        wv2_sb = persist.tile([2 * C, C], F16)
        bpc_sb = persist.tile([C, 1], F32)
        c2_sb = persist.tile([2 * C, 1], F32)
        pair_sb = persist.tile([C, GROUPS], F32)
        expand2_sb = persist.tile([GROUPS, 2 * C], F32)
        stats_sb = persist.tile([C, 6 * nch], F32)
        scratch_sb = persist.tile([C, sch], F32)
        scratch2_sb = persist.tile([C, sch], F32)
        eps_sb = persist.tile([GROUPS, 1], F32)
        mrg_sb = persist.tile([GROUPS, 3], F32)      # [mean, rstd, sd]
        mrc2_sb = persist.tile([2 * C, 3], F32)
        tmpc_sb = persist.tile([2 * C, 1], F32)
        warm_sb = persist.tile([NT, 512], F16)
        actscr = persist.tile([GROUPS, 1], F32)

        # ---- PE warm-up burst + ACT table preloads at t=0 ----
        nc.vector.memset(warm_sb[:], 0.25)
        nc.vector.memset(eps_sb[:], EPS)
        for _ in range(10):
            warm_ps = prodpool.tile([NT, 512], F32, tag="prod", name="warm_ps")
            nc.tensor.matmul(
                warm_ps[:], warm_sb[:, 0:NT], warm_sb[:], start=True, stop=True,
            )
        # the ONLY ACT table set (exp; Copy rides in every set) loads at
        # t=0 inside the DMA shadow
        nc.scalar.activation(out=actscr[:], in_=eps_sb[:], func=AF.Exp)

        # DMA priority: first few x chunks (gate the stats pipeline), then
        # the small constants (the group-stats matmul needs pair_sb early;
        # queued after the full 4MB x load it arrived ~25us late), then the
        # rest of x; the bottom-half duplicate (for the partition-aligned
        # bottom normalize) is only needed ~15us later
        for ch in range(4):
            sl = slice(ch * sch, (ch + 1) * sch)
            nc.sync.dma_start(out=xb2_sb[0:C, sl], in_=xb_d[:, sl])
        nc.sync.dma_start(out=pair_sb[:], in_=pair_d[:, :])
        nc.sync.dma_start(out=expand2_sb[:], in_=expand2_d[:, :])
        nc.sync.dma_start(out=wq4_sb[:], in_=wq4_d[:, :])
        nc.sync.dma_start(out=wk4_sb[:], in_=wk4_d[:, :])
        nc.sync.dma_start(out=wv2_sb[:], in_=wv2_d[:, :])
        nc.sync.dma_start(out=bpc_sb[:], in_=bpc_d[:, :])
        nc.sync.dma_start(out=c2_sb[:], in_=c2_d[:, :])
        for ch in range(4, nch):
            sl = slice(ch * sch, (ch + 1) * sch)
            nc.sync.dma_start(out=xb2_sb[0:C, sl], in_=xb_d[:, sl])
        for ch in range(xch):
            sl = slice(ch * xsz, (ch + 1) * xsz)
            nc.sync.dma_start(out=xb2_sb[C : 2 * C, sl], in_=xb_d[:, sl])
        # ones column (col C of each 65-wide v^T block) -> AV rowsum
        nc.gpsimd.memset(vt_sb[:], 1.0)

        # ---- per-channel mean/var in one DVE pass per chunk ----
        for ch in range(nch):
            sl = slice(ch * sch, (ch + 1) * sch)
            nc.vector.bn_stats(
                out=stats_sb[:, ch * 6 : (ch + 1) * 6], in_=xb2_sb[0:C, sl])
        mv = mpool.tile([C, 2], F32, tag="mv")
        nc.vector.bn_aggr(
            out=mv[:], in_=stats_sb[:].rearrange("p (c s) -> p c s", s=6))
        # E2 = var + mean^2 so the group merge stays linear in channel stats
        msqc = mpool.tile([C, 1], F32, tag="msqc")
        nc.vector.tensor_mul(msqc[:], mv[:, 0:1], mv[:, 0:1])
        nc.vector.tensor_add(mv[:, 1:2], mv[:, 1:2], msqc[:])

        # ---- group statistics: channel-pair averages via matmul ----
        gp = prodpool.tile([GROUPS, 2], F32, tag="prod")
        nc.tensor.matmul(gp[:], pair_sb[:], mv[:], start=True, stop=True)
        gsum = mpool.tile([GROUPS, 2], F32, tag="gsum")
        nc.vector.tensor_copy(gsum[:], gp[:])
        msq = mpool.tile([GROUPS, 1], F32, tag="msq")
        nc.vector.tensor_mul(msq[:], gsum[:, 0:1], gsum[:, 0:1])
        nc.vector.tensor_copy(mrg_sb[:, 0:1], gsum[:, 0:1])
        # vpe = (Ex2 + eps) - mean^2
        vpe = mpool.tile([GROUPS, 1], F32, tag="vpe")
        nc.vector.scalar_tensor_tensor(
            out=vpe[:], in0=gsum[:, 1:2], scalar=eps_sb[:], in1=msq[:],
            op0=OP.add, op1=OP.subtract,
        )
        # rstd = rsqrt(vpe) via Newton from y0 = 1/vpe (exact for var ~= 1;
        # 3 iterations cover var in [0.3, 3] to fp32); avoids the ACT sqrt
        # table-set load entirely
        y = mpool.tile([GROUPS, 1], F32, tag="ynew")
        t = mpool.tile([GROUPS, 1], F32, tag="tnew")
        nc.vector.reciprocal(y[:], vpe[:])
        for _ in range(2):
            nc.vector.tensor_mul(t[:], y[:], y[:])
            nc.vector.tensor_mul(t[:], t[:], vpe[:])
            nc.vector.tensor_scalar(
                out=t[:], in0=t[:], scalar1=-0.5, scalar2=1.5,
                op0=OP.mult, op1=OP.add,
            )
            nc.vector.tensor_mul(y[:], y[:], t[:])
        nc.vector.tensor_copy(mrg_sb[:, 1:2], y[:])
        # sd = vpe * rstd (= sqrt(vpe)), used only by the bq c-fold
        nc.vector.tensor_mul(mrg_sb[:, 2:3], vpe[:], y[:])
        ep = prodpool.tile([2 * C, 3], F32, tag="prod")
        nc.tensor.matmul(ep[:], expand2_sb[:], mrg_sb[:], start=True, stop=True)
        nc.vector.tensor_copy(mrc2_sb[:], ep[:])
        # bq c-fold: adjusted mean m' = m - c*sd so (x-m')*r = xn + c
        nc.vector.tensor_mul(tmpc_sb[:], c2_sb[:], mrc2_sb[:, 2:3])
        nc.vector.tensor_sub(mrc2_sb[:, 0:1], mrc2_sb[:, 0:1], tmpc_sb[:])

        # ---- production: normalize + projections (pull-scheduled) ----
        vt_view = vt_sb[:].rearrange("p (t e) -> p t e", e=C + 1)

        def emit_xh(ch, sl=None):
            if sl is None:
                sl = slice(ch * xsz, (ch + 1) * xsz)
            for h in (0, 1):
                hs = slice(h * C, (h + 1) * C)
                nc.vector.tensor_scalar(
                    out=xh2_sb[hs, sl], in0=xb2_sb[hs, sl],
                    scalar1=mrc2_sb[hs, 0:1], scalar2=mrc2_sb[hs, 1:2],
                    op0=OP.subtract, op1=OP.mult,
                )

        def proj_q(j):
            sl = slice(j * PCH, (j + 1) * PCH)
            qp = prodpool.tile([2 * C, PCH], F32, tag="prod", name="qp")
            nc.tensor.matmul(qp[:], wq4_sb[:], xh2_sb[:, sl], start=True, stop=True)
            nc.vector.tensor_copy(q2_sb[:, sl], qp[:])

        def proj_k(j):
            # chunk j = tiles 4j..4j+3 = pairs 2j, 2j+1; pack even tiles into
            # the top k2 half, odd into the bottom (partition-aligned copies)
            sl = slice(j * PCH, (j + 1) * PCH)
            pool = spool if j < 2 else prodpool
            kp = pool.tile([2 * C, PCH], F32, tag="prod" if j >= 2 else "s",
                           name="kp")
            nc.tensor.matmul(kp[:], wk4_sb[:], xh2_sb[:, sl], start=True, stop=True)
            kv = kp[:].rearrange("p (a h c) -> p a h c", a=2, h=2)
            for h in (0, 1):
                hs = slice(h * C, (h + 1) * C)
                dst = k2_sb[hs, 2 * j * NT : (2 * j + 2) * NT]
                nc.vector.tensor_copy(
                    dst.rearrange("p (a c) -> p a c", a=2), kv[hs, :, h, :],
                )

        def proj_vt(b):
            # batch b = tiles 8b..8b+7, one matmul each, single strided copy
            t0 = b * 8
            vp = prodpool.tile([NT, 8 * C], F32, tag="prod", name="vp")
            for t in range(8):
                nc.tensor.matmul(
                    vp[:, t * C : (t + 1) * C],
                    xh2_sb[:, (t0 + t) * NT : (t0 + t + 1) * NT],
                    wv2_sb[:], start=True, stop=True,
                )
            nc.vector.tensor_copy(
                vt_view[:, t0 : t0 + 8, 0:C],
                vp[:].rearrange("p (t e) -> p t e", e=C),
            )

        steps = []
        qsteps = []
        for c in range(xch):
            base_k, base_v = 4 * c, 2 * c
            if c == 0:
                # fine-grained: normalize just enough for each projection so
                # the first scores (and exps) start ~5us earlier
                steps.append(("xhs", slice(0, PCH)))
                steps.append(("q", 0))
                qsteps.extend(("q", j) for j in range(1, m_tok // PCH))
                steps.append(("k", 0))
                steps.append(("xhs", slice(PCH, 2 * PCH)))
                steps.append(("k", 1))
                steps.append(("xhs", slice(2 * PCH, 3 * PCH)))
                steps.append(("xhs", slice(3 * PCH, 4 * PCH)))
                steps.append(("vt", 0))
                steps.append(("k", 2))
                steps.append(("k", 3))
                steps.append(("vt", 1))
                continue
            steps.append(("xh", c))
            steps.append(("k", base_k))
            steps.append(("k", base_k + 1))
            steps.append(("vt", base_v))
            steps.append(("k", base_k + 2))
            steps.append(("k", base_k + 3))
            steps.append(("vt", base_v + 1))

        state = {"pairs": 0, "vt": 0, "q": 0}

        def pull(need_pairs=0, need_vt=0):
            while steps and (state["pairs"] < need_pairs or state["vt"] < need_vt):
                kind, arg = steps.pop(0)
                if kind == "xh":
                    emit_xh(arg)
                elif kind == "xhs":
                    emit_xh(0, sl=arg)
                elif kind == "q":
                    proj_q(arg)
                    state["q"] += 1
                elif kind == "k":
                    proj_k(arg)
                    state["pairs"] += 2
                else:
                    proj_vt(arg)
                    state["vt"] += 8

        def pull_q(need):
            while qsteps and state["q"] < need:
                _, arg = qsteps.pop(0)
                proj_q(arg)
                state["q"] += 1

        # ---- attention: per 256-query block, row-tiled score pairs ->
        # fp32 PSUM -> exp (fp16) -> col-tiled AV pairs + 4x col-tiled
        # ones-matmul rowsums, accumulated in one shared PSUM bank ----
        def make_tail(av_sb, msl, last):
            def tail():
                recip = mpool.tile([1, MB], F32, tag="recip", name="recip")
                nc.vector.reciprocal(recip[:], av_sb[C : C + 1, :])
                rb = mpool.tile([C, MB], F32, tag="rb", name="rb")
                nc.gpsimd.partition_broadcast(rb[:], recip[:], channels=C)
                t1 = mpool.tile([C, MB], F32, tag="t1", name="t1")
                nc.vector.tensor_mul(t1[:], av_sb[0:C, :], rb[:])
                outt = mpool.tile([C, MB], F32, tag="outt", name="outt")
                nc.vector.scalar_tensor_tensor(
                    out=outt[:], in0=t1[:], scalar=bpc_sb[:], in1=xb2_sb[0:C, msl],
                    op0=OP.add, op1=OP.add,
                )
                if last:
                    h = MB // 4
                    for u in range(4):
                        nc.sync.dma_start(
                            out=out_d[:, msl.start + u * h : msl.start + (u + 1) * h],
                            in_=outt[:, u * h : (u + 1) * h])
                else:
                    nc.sync.dma_start(out=out_d[:, msl], in_=outt[:])
            return tail

        deferred = None
        deferred_av = None
        for mb in range(nblk):
            msl = slice(mb * MB, (mb + 1) * MB)
            if mb >= 2:
                pull_q(mb // 2 + 1)
            av = avpool.tile([C + 1, MB], F32, tag="av")
            exmap = {}
            pend = None

            def do_av_rs(g, exmap=exmap, av=av, mb=mb):
                gsz = gsize(g)
                if mb == 0:
                    pull(need_vt=min(g * GSZ + gsz, ntiles))
                ex = exmap[g]
                half = gsz // 2
                first = (g == 0)
                last = (g == ngroups - 1)
                for s in range(gsz):
                    # slot permutation (see scores loop): slot s holds tile
                    # 2*(g*GSZ//2 + s%half) + s//half
                    t = 2 * ((g * GSZ) // 2 + s % half) + s // half
                    nc.tensor.matmul(
                        av[:], vt_view[:, t, :],
                        ex[:, s * MB : (s + 1) * MB],
                        start=(first and s == 0), stop=(last and s == gsz - 1),
                    )

            for g in range(ngroups):
                gsz = gsize(g)
                sp = spool.tile([NT, gsz * MB], F32, tag="s")
                half = gsz // 2
                for i in range(half):
                    p = (g * GSZ) // 2 + i
                    if mb == 0:
                        pull(need_pairs=min(p + 2, npairs))
                    for h in (0, 1):
                        # the two concurrent row-tiled halves MUST write
                        # different PSUM banks (same-bank concurrent PE
                        # writes fault the exec unit -- measured); slot
                        # i + half*h puts them 1.5 banks apart
                        s = i + half * h
                        nc.tensor.matmul(
                            sp[:, s * MB : (s + 1) * MB],
                            k2_sb[h * C : (h + 1) * C, p * NT : (p + 1) * NT],
                            q2_sb[h * C : (h + 1) * C, msl],
                            start=True, stop=True,
                        )
                ex = epool.tile([NT, gsz * MB], F16, tag="e")
                nc.scalar.activation(out=ex[:], in_=sp[:], func=AF.Exp)
                exmap[g] = ex
                if g == 1 and deferred_av is not None:
                    # previous block's last AV group + accumulator
                    # evacuation, deferred so this block's first scores
                    # (and exps) aren't stuck behind the PE's AV tail
                    deferred_av()
                    deferred_av = None
                if pend is not None:
                    do_av_rs(pend)
                pend = g
                if g == 4 and deferred is not None:
                    deferred()
                    deferred = None
            last_pend = pend

            def finish_block(do_av_rs_f, av_t, g, msl_, is_last):
                # explicit capture: do_av_rs/av/exmap are rebound per block
                def fin():
                    nonlocal deferred
                    do_av_rs_f(g)
                    av_sb = mpool.tile(
                        [C + 1, MB], F32, tag="avsb", name="av_sb")
                    nc.vector.tensor_copy(av_sb[:], av_t[:])
                    if deferred is not None:
                        deferred()
                    deferred = make_tail(av_sb, msl_, last=is_last)
                return fin

            deferred_av = finish_block(do_av_rs, av, last_pend, msl, mb == nblk - 1)
            if mb == nblk - 1:
                deferred_av()
                deferred_av = None
        deferred()


def build_program(n_tok=N_FULL, m_tok=M_FULL):
    nc = bacc.Bacc("TRN2", target_bir_lowering=False, debug=False)
    xb_d = nc.dram_tensor("xb", [C, n_tok], F32, kind="ExternalInput")
    wq4_d = nc.dram_tensor("wq4", [2 * C, 2 * C], F16, kind="ExternalInput")
    wk4_d = nc.dram_tensor("wk4", [2 * C, 2 * C], F16, kind="ExternalInput")
    wv2_d = nc.dram_tensor("wv2", [2 * C, C], F16, kind="ExternalInput")
    bpc_d = nc.dram_tensor("bpc", [C, 1], F32, kind="ExternalInput")
    c2_d = nc.dram_tensor("c2", [2 * C, 1], F32, kind="ExternalInput")
    pair_d = nc.dram_tensor("pair", [C, GROUPS], F32, kind="ExternalInput")
    expand2_d = nc.dram_tensor("expand2", [GROUPS, 2 * C], F32, kind="ExternalInput")
    out_d = nc.dram_tensor("out", [C, m_tok], F32, kind="ExternalOutput")
    with tile.TileContext(nc) as tc:
        emit(tc, nc, n_tok, m_tok,
             xb_d.ap(), wq4_d.ap(), wk4_d.ap(), wv2_d.ap(), bpc_d.ap(),
             c2_d.ap(), pair_d.ap(), expand2_d.ap(), out_d.ap())
    nc.compile()
    return nc


def _split16(a):
    hi = a.astype(np.float16)
    lo = (a - hi.astype(np.float32)).astype(np.float16)
    return hi, lo


def prep_weights(gamma, beta, wq, bq, wk, bk, wv, bv, wp, bp, n_tok=N_FULL):
    """Host-side algebraic folds. Returns the shared per-core input dict."""
    f32 = np.float32
    gamma, beta = gamma.astype(f32), beta.astype(f32)
    scale = f32(1.0) / np.sqrt(f32(C)).astype(f32)
    wq_eff = (wq * gamma[None, :]) * scale
    bq_eff = (wq @ beta + bq) * scale
    wk_eff = wk * gamma[None, :]
    wv_eff = wv * gamma[None, :]
    bv_eff = wv @ beta + bv
    wpv_eff = (wp @ wv_eff).astype(f32)

    if np.abs(bq_eff).max() > 0:
        c = np.linalg.lstsq(wq_eff, bq_eff, rcond=None)[0].astype(f32)
    else:
        c = np.zeros(C, f32)
    bp_eff = (bp + wp @ bv_eff - wpv_eff @ c).astype(f32)

    pair = np.zeros((C, GROUPS), f32)
    pair[np.arange(C), np.arange(C) // 2] = f32(0.5)
    expand2 = np.zeros((GROUPS, 2 * C), f32)
    expand2[np.arange(2 * C) % C // 2, np.arange(2 * C)] = 1.0

    def stack16(a):
        hi, lo = _split16(np.ascontiguousarray(a, f32))
        return np.ascontiguousarray(np.concatenate([hi, lo], axis=0))

    def dup_cols(a):  # [128, 64] -> [128, 128]
        return np.ascontiguousarray(np.concatenate([a, a], axis=1))

    return {
        "wq4": dup_cols(stack16(wq_eff.T)),
        "wk4": dup_cols(stack16(wk_eff.T)),
        "wv2": stack16(wpv_eff.T),
        "bpc": bp_eff.reshape(C, 1),
        "c2": np.ascontiguousarray(np.concatenate([c, c]).reshape(2 * C, 1)),
        "pair": pair,
        "expand2": expand2,
    }


_PROGRAM_CACHE = {}


def _get_program(n_tok, m_tok):
    key = (n_tok, m_tok)
    if key not in _PROGRAM_CACHE:
        _PROGRAM_CACHE[key] = build_program(n_tok, m_tok)
    return _PROGRAM_CACHE[key]


def make_in_maps(x, shared):
    """Per-core input maps: batch b = core//4, query chunk qc = core%4."""
    in_maps = []
    for core in range(N_CORES):
        b, qc = core // Q_CHUNKS, core % Q_CHUNKS
        xb = np.ascontiguousarray(x[b].reshape(C, N_FULL), np.float32)
        xb = np.ascontiguousarray(np.roll(xb, -qc * M_FULL, axis=1))
        in_maps.append({"xb": xb, **shared})
    return in_maps


def kernel(x, gamma, beta, wq, bq, wk, bk, wv, bv, wp, bp, **run_kwargs):
    from concourse.bass_utils import run_bass_kernel_spmd

    x = np.asarray(x, np.float32)
    shared = prep_weights(
        np.asarray(gamma), np.asarray(beta), np.asarray(wq), np.asarray(bq),
        np.asarray(wk), np.asarray(bk), np.asarray(wv), np.asarray(bv),
        np.asarray(wp), np.asarray(bp),
    )
    nc = _get_program(N_FULL, M_FULL)
    in_maps = make_in_maps(x, shared)
    res = run_bass_kernel_spmd(nc, in_maps, core_ids=list(range(N_CORES)), **run_kwargs)
    y = np.empty((B_FULL, C, N_FULL), np.float32)
    for core in range(N_CORES):
        b, qc = core // Q_CHUNKS, core % Q_CHUNKS
        y[b, :, qc * M_FULL : (qc + 1) * M_FULL] = res.results[core]["out"]
    out = y.reshape(B_FULL, C, H_FULL, W_FULL, D_FULL)
    if run_kwargs:
        return out, res
    return out
